# revision 17
# baseline (speedup 1.0000x reference)
"""Trainium2 Bass kernel for ragged-sequence attention.

reference computation (per batch b):
    energy[s] = sum_d key[s,b,:] . query[b,:]          (S=2048, B=32, D=512)
    w = softmax(energy) * mask;  w /= max(sum(w), eps)
    context = sum_s w[s] * value[s,b,:]
returns (context (B,D), w (B,S)).

Sharding: data parallel over batch, 4 batches per core on 8 cores, with a
rank-balanced batch->core assignment that equalizes ragged traffic:
batches sorted by tile count TB=ceil(len/128); core c gets ranks
{c, 8+c, 16+c, 24+c}; slot k's compiled tile count is Q[k] = TB[rank 8k].

Raggedness:
  - V: w is exactly zero for s >= len_b, so V rows beyond 128*TB[b] never
    contribute.  Only Q[k] V tiles are loaded per slot.
  - K: the reference normalizes by max(sum(w_masked), eps) with
    eps = 1e-12.  Whenever that clamp does NOT fire, the softmax max and
    the full-row normalizer cancel algebraically, so only the first
    TB[b] K tiles affect the output.  A host-side fp64 guard checks
    sigma = masked softmax mass > 1.5e-12 for every batch (the clamp
    fires at 1e-12; sigma's fp uncertainty is ~1e-5 relative, so the
    margin is decisive).  If any batch is too close, the kernel falls
    back to a full-K program that reproduces the reference exactly,
    including the clamp (denominator max(S_m, eps*Z)).
  - Since Q is non-increasing, the slots needing tile t are always a
    prefix: K and V load one prefix-packed tile [128p, a(t) slots, 512d]
    per t (contiguous 2-8 KiB runs).

Per-core dataflow:
  - q arrives pre-broadcast from the host ([128, 4*512], first on the ring).
  - energy via fused multiply+reduce (affine_mul_reduce) on DVE.
  - softmax: critical path to the context is only max (DVE reduce +
    gpsimd partition_all_reduce) -> exp (ACT, fused row sums) -> mask
    (DVE fused mask+mult+sum) -> PE transpose.  The normalizer
    r = 1/max(S_m, eps*Z) is computed off-path and folded into the final
    context copy (ACT scale) and the w output scale.
  - context: fp32 skinny matmuls over unnormalized weights, scheduled
    across the 4 PE column groups (tile_position) so up to 4 accumulation
    chains run concurrently; split-slot partials merged at the end.
  - w rows beyond the loaded K tiles stay zero via the runtime's
    zero-initialized output buffers, matching the reference mask.
"""
import contextlib
import os

import numpy as np

S, B, D = 2048, 32, 512
NCORES = 8
BL = B // NCORES          # 4 local batches (slots) per core
P = 128                   # partitions
NT = S // P               # 16 s-tiles of 128
EPS = 1e-12

_CACHE = {}
LAST_RESULT = None


def _chain_assignment(Q):
    """Assign each slot's context matmuls to PE column groups.  A slot
    larger than the balanced share is split across two groups; others get
    one group (no merge needed)."""
    total = sum(Q)
    share = (total + BL - 1) // BL
    loads = [0] * BL
    chains = {}          # slot -> list of (group, t_list)
    for s in sorted(range(BL), key=lambda x: -Q[x]):
        ts = list(range(Q[s]))
        if Q[s] > share and len(ts) >= 2:
            h = (Q[s] + 1) // 2
            parts = [ts[:h], ts[h:]]
        else:
            parts = [ts]
        lst = []
        for part in parts:
            g = min(range(BL), key=lambda x: loads[x])
            loads[g] += len(part)
            lst.append((g, part))
        chains[s] = lst
    return chains


def _build(Q, QK):
    from concourse import bacc, mybir
    import concourse.tile as tile
    from concourse import bass_isa

    F32 = mybir.dt.float32
    AX = mybir.AxisListType.X
    ALU = mybir.AluOpType
    EXP = mybir.ActivationFunctionType.Exp

    def nka(t):
        return sum(1 for s in range(BL) if t < QK[s])

    def nva(t):
        return sum(1 for s in range(BL) if t < Q[s])

    nc = bacc.Bacc("TRN2", target_bir_lowering=False, debug=False,
                   num_devices=NCORES)

    K_in = nc.dram_tensor("K_in", [S, BL, D], F32, kind="ExternalInput").ap()
    V_in = nc.dram_tensor("V_in", [S, BL, D], F32, kind="ExternalInput").ap()
    qb_in = nc.dram_tensor("qb_in", [P, BL * D], F32, kind="ExternalInput").ap()
    th_in = nc.dram_tensor("th_in", [NT, BL], F32, kind="ExternalInput").ap()
    io_in = nc.dram_tensor("io_in", [NT, P], F32, kind="ExternalInput").ap()
    id_in = nc.dram_tensor("id_in", [P, P], F32, kind="ExternalInput").ap()
    ctx_out = nc.dram_tensor("ctx_out", [1, BL * D], F32,
                             kind="ExternalOutput").ap()
    w_out = nc.dram_tensor("w_out", [BL, NT, P], F32, kind="ExternalOutput").ap()

    # prefix-packed tiles: tile t = [128p, a(t) slots, 512d]
    Kv = K_in.rearrange("(t p) b d -> t p b d", t=NT, p=P)
    Vv = V_in.rearrange("(t p) b d -> t p b d", t=NT, p=P)

    chains = _chain_assignment(Q)

    with tile.TileContext(nc) as tc:
        with contextlib.ExitStack() as ctx:
            kp = ctx.enter_context(tc.tile_pool(name="kp", bufs=1))
            vp = ctx.enter_context(tc.tile_pool(name="vp", bufs=1))
            cb = ctx.enter_context(tc.tile_pool(name="cb", bufs=1))
            tp = ctx.enter_context(tc.tile_pool(name="tp", bufs=2))
            wp = ctx.enter_context(tc.tile_pool(name="wp", bufs=4))
            pp = ctx.enter_context(tc.tile_pool(name="pp", bufs=1, space="PSUM"))

            # --- q (pre-broadcast) + constants first on the sync ring ---
            qbsb = cb.tile([P, BL * D], F32)
            nc.sync.dma_start(qbsb[:], qb_in[:])
            ones = cb.tile([NT, P], F32)
            nc.gpsimd.memset(ones[:], 1.0)

            # --- big-stream DMAs: ragged K then ragged V (all resident),
            # consecutive same-width tiles coalesced into <=1MB chunks ---
            def chunks(afn):
                out, t = [], 0
                while t < NT:
                    a = afn(t)
                    if a == 0:
                        break
                    n = 1
                    while (t + n < NT and afn(t + n) == a
                           and (n + 1) * a * 256 * 1024 <= 1 << 20):
                        n += 1
                    out.append((t, n, a))
                    t += n
                return out

            kchunks, vchunks = chunks(nka), chunks(nva)

            def load(pool, view, chlist, kind):
                tiles = [None] * NT
                for idx, (t0c, n, a) in enumerate(chlist):
                    tl = pool.tile([P, n, a, D], F32, name=f"{kind}{idx}",
                                   tag=f"{kind}{idx}", bufs=1)
                    src_ap = view[t0c * P:(t0c + n) * P, 0:a, :].rearrange(
                        "(n p) b d -> p n b d", p=P)
                    nc.sync.dma_start(tl[:], src_ap)
                    for j in range(n):
                        tiles[t0c + j] = tl[:, j, :, :]
                return tiles

            ktiles = load(kp, K_in, kchunks, "kc")
            vtiles = load(vp, V_in, vchunks, "vc")

            # softmax-phase constants (needed ~mid-kernel)
            ident = cb.tile([P, P], F32)
            nc.sync.dma_start(ident[:], id_in[:])
            iota = cb.tile([NT, P], F32)
            nc.sync.dma_start(iota[:], io_in[:])
            th = cb.tile([NT, BL], F32)
            nc.sync.dma_start(th[:], th_in[:])

            # --- energy: E_b[:, t] = sum_d K[s,:]*q (fused DVE), K-paced ---
            Es = [cb.tile([P, QK[b] + 1], F32, tag=f"E{b}", name=f"E{b}")
                  for b in range(BL)]
            for t in range(NT):
                for b in range(BL):
                    if t < QK[b]:
                        tmp = tp.tile([P, D], F32, tag="tmp")
                        nc.vector.affine_mul_reduce(
                            out=tmp[:], accum_out=Es[b][:, t:t + 1],
                            in0=ktiles[t][:, b, :],
                            in1=qbsb[:, b * D:(b + 1) * D],
                            scale=1.0, bias=0.0)

            # --- per-slot softmax (shapes sized to QK[b]) ---
            wTs, r1s = [], []
            for b in range(BL):
                E = Es[b]
                n = QK[b]
                # critical path: max -> exp -> mask -> transpose
                nc.vector.reduce_max(E[:, n:n + 1], E[:, 0:n], axis=AX)
                ar = wp.tile([P, 1], F32, tag="ar")
                nc.gpsimd.partition_all_reduce(
                    ar[:], E[:, n:n + 1], channels=P,
                    reduce_op=bass_isa.ReduceOp.max)
                nm = wp.tile([n, 1], F32, tag="nm", name="nm")
                nc.vector.tensor_scalar_mul(nm[:], ar[0:n, :], -1.0)

                Et_ps = pp.tile([n, P], F32, tag="Et_ps", name="Et_ps")
                nc.tensor.transpose(Et_ps[:], E[:, 0:n], ident[:])
                Et = wp.tile([n, P], F32, tag="Et", name="Et")
                nc.scalar.copy(Et[:], Et_ps[:])

                X = wp.tile([n, P], F32, tag="X", name="X")
                scat = wp.tile([n, 2], F32, tag="scat", name="scat")
                nc.scalar.activation(X[:], Et[:], EXP, bias=nm[:], scale=1.0,
                                     accum_out=scat[:, 0:1])

                W1 = wp.tile([n, P], F32, tag=f"W1{b}", name=f"W1{b}")
                nc.vector.scalar_tensor_tensor(
                    out=W1[:], in0=iota[0:n, :], scalar=th[0:n, b:b + 1],
                    in1=X[:], op0=ALU.is_lt, op1=ALU.mult,
                    accum_out=scat[:, 1:2])

                wT_ps = pp.tile([P, n], F32, tag="wT_ps", name="wT_ps")
                nc.tensor.transpose(wT_ps[:], W1[:], ident[0:n, 0:n])
                wT = wp.tile([P, n], F32, tag=f"wT{b}", name=f"wT{b}")
                nc.scalar.copy(wT[:], wT_ps[:])
                wTs.append(wT)

                # off path: normalizer r = 1/max(S_m, eps*Z), w output
                ss_ps = pp.tile([1, 2], F32, tag="sm_ps", name="ss_ps")
                nc.tensor.matmul(ss_ps[:], ones[0:n, 0:1], scat[:],
                                 start=True, stop=True)
                ss = wp.tile([1, 2], F32, tag="ss")
                nc.scalar.copy(ss[:], ss_ps[:])
                d1 = wp.tile([1, 1], F32, tag="d1")
                nc.vector.tensor_scalar_mul(d1[:], ss[:, 0:1], EPS)
                d2 = wp.tile([1, 1], F32, tag="d2")
                nc.vector.tensor_scalar_max(d2[:], ss[:, 1:2], d1[:])
                r1 = wp.tile([1, 1], F32, tag=f"r1{b}", name=f"r1{b}")
                nc.vector.reciprocal(r1[:], d2[:])
                r1s.append(r1)
                r_ps = pp.tile([n, 1], F32, tag="sm_ps", name="r_ps")
                nc.tensor.matmul(r_ps[:], ones[0:1, 0:n], r1[:],
                                 start=True, stop=True)
                r16 = wp.tile([n, 1], F32, tag="r16", name="r16")
                nc.scalar.copy(r16[:], r_ps[:])
                W = wp.tile([n, P], F32, tag=f"W{b}", name=f"W{b}")
                nc.vector.tensor_scalar_mul(W[:], W1[:], r16[:])
                nc.sync.dma_start(w_out[b, 0:n], W[:])

            # --- context: chains across PE column groups ---
            cps = [pp.tile([P, D], F32, tag="ctx_ps", bufs=BL, name=f"cps{b}")
                   for b in range(BL)]
            group_of, first_last = {}, {}
            for s in range(BL):
                for g, ts in chains[s]:
                    for t in ts:
                        group_of[(s, t)] = g
                    first_last[(s, g)] = (ts[0], ts[-1])
            for t in range(NT):
                for s in range(BL):
                    if t < Q[s]:
                        g = group_of[(s, t)]
                        ft, lt = first_last[(s, g)]
                        nc.tensor.matmul(
                            cps[s][32 * g:32 * g + 1, :],
                            wTs[s][:, t:t + 1], vtiles[t][:, s, :],
                            start=(t == ft), stop=(t == lt),
                            tile_position=(0, 32 * g))

            # merge partial chains, scale by r, single output DMA
            cs_row = cb.tile([1, BL * D], F32)
            for s in range(BL):
                gl = chains[s]
                dst = cs_row[0:1, s * D:(s + 1) * D]
                if len(gl) == 1:
                    g = gl[0][0]
                    nc.scalar.mul(dst, cps[s][32 * g:32 * g + 1, :], r1s[s][:])
                else:
                    gA, gB = gl[0][0], gl[1][0]
                    mgB = wp.tile([1, D], F32, tag=f"mgB{s}", name=f"mgB{s}")
                    nc.scalar.copy(mgB[:], cps[s][32 * gB:32 * gB + 1, :])
                    mg = wp.tile([1, D], F32, tag=f"mg{s}", name=f"mg{s}")
                    nc.vector.tensor_add(mg[:], cps[s][32 * gA:32 * gA + 1, :],
                                         mgB[:])
                    nc.scalar.mul(dst, mg[:], r1s[s][:])
            nc.sync.dma_start(ctx_out[:], cs_row[:])

    nc.compile()
    return nc


def _get_nc(Q, QK):
    key = (Q, QK)
    if key not in _CACHE:
        _CACHE[key] = _build(Q, QK)
    return _CACHE[key]


def _maybe_register_trace_hook():
    """run_bass_kernel_spmd(trace=True) under axon needs antenv.axon_hooks;
    the RL image lacks it, so synthesize the module from trn_agent_boot."""
    import sys, types
    if "antenv.axon_hooks" in sys.modules:
        return
    try:
        import trn_agent_boot.trn_boot as tb
        hook = tb._ntff_profile_via_ctypes('/opt/axon/libaxon_pjrt.so')
        mod = types.ModuleType('antenv.axon_hooks')
        mod.get_axon_ntff_profile_hook = lambda: hook
        mod.set_axon_ntff_profile_hook = lambda h: None
        sys.modules['antenv.axon_hooks'] = mod
    except Exception:
        pass


def _clamp_guard_ok(key, query, lens):
    """True iff no batch's masked softmax mass is close enough to the
    reference's 1e-12 clamp for the clamp decision to be in doubt."""
    e = np.matmul(key.transpose(1, 0, 2), query[:, :, None])[:, :, 0]  # (B,S)
    M = e.max(axis=1, keepdims=True)
    ex = np.exp((e - M).astype(np.float64))
    sigma = (ex * (np.arange(S)[None, :] < lens[:, None])).sum(axis=1) \
        / ex.sum(axis=1)
    return bool((sigma > 1.5e-12).all())


def kernel(key, value, query, encoder_condensed_lens):
    global LAST_RESULT
    key = np.asarray(key, dtype=np.float32)
    value = np.asarray(value, dtype=np.float32)
    query = np.asarray(query, dtype=np.float32)
    lens = np.asarray(encoder_condensed_lens).astype(np.int64).reshape(B)

    # rank-balanced assignment: sort batches by V tile count (desc); core c
    # slot k <- rank 8k+c.  Q[k] = max tile count within slot k = rank 8k.
    TB = np.clip(np.ceil(lens / P).astype(int), 1, NT)
    order = np.argsort(-TB, kind="stable")
    Q = tuple(int(TB[order[8 * k]]) for k in range(BL))
    QK = Q if _clamp_guard_ok(key, query, lens) else (NT,) * BL

    iota_np = np.tile(np.arange(P, dtype=np.float32)[None, :], (NT, 1))
    ident_np = np.eye(P, dtype=np.float32)

    in_maps = []
    batch_of = np.empty((NCORES, BL), dtype=int)
    for c in range(NCORES):
        bsel = [int(order[8 * k + c]) for k in range(BL)]
        batch_of[c] = bsel
        th = np.empty((NT, BL), dtype=np.float32)
        for k in range(BL):
            th[:, k] = lens[bsel[k]] - P * np.arange(NT)
        qb = np.tile(query[bsel, :].reshape(1, BL * D), (P, 1))
        in_maps.append({
            "K_in": np.ascontiguousarray(key[:, bsel, :]),
            "V_in": np.ascontiguousarray(value[:, bsel, :]),
            "qb_in": qb,
            "th_in": th,
            "io_in": iota_np,
            "id_in": ident_np,
        })

    nc = _get_nc(Q, QK)
    from concourse import bass_utils
    trace = bool(os.environ.get("BASS_TRACE"))
    if trace:
        _maybe_register_trace_hook()
    res = bass_utils.run_bass_kernel_spmd(
        nc, in_maps, core_ids=list(range(NCORES)), trace=trace)
    LAST_RESULT = res

    context = np.empty((B, D), dtype=np.float32)
    w = np.empty((B, S), dtype=np.float32)
    for c in range(NCORES):
        r = res.results[c]
        ctxc = r["ctx_out"].reshape(BL, D)
        for k in range(BL):
            context[batch_of[c][k]] = ctxc[k]
            w[batch_of[c][k]] = r["w_out"][k].reshape(S)
    return context, w


# revision 18
# speedup vs baseline: 1.0498x; 1.0498x over previous
"""Trainium2 Bass kernel for ragged-sequence attention.

reference computation (per batch b):
    energy[s] = sum_d key[s,b,:] . query[b,:]          (S=2048, B=32, D=512)
    w = softmax(energy) * mask;  w /= max(sum(w), eps)
    context = sum_s w[s] * value[s,b,:]
returns (context (B,D), w (B,S)).

Sharding: data parallel over batch, 4 batches per core on 8 cores, with a
rank-balanced batch->core assignment that equalizes ragged traffic:
batches sorted by tile count TB=ceil(len/128); core c gets ranks
{c, 8+c, 16+c, 24+c}; slot k's compiled tile count is Q[k] = TB[rank 8k].

Raggedness:
  - V: w is exactly zero for s >= len_b, so V rows beyond 128*TB[b] never
    contribute.  Only Q[k] V tiles are loaded per slot.
  - K: the reference normalizes by max(sum(w_masked), eps) with
    eps = 1e-12.  Whenever that clamp does NOT fire, the softmax max and
    the full-row normalizer cancel algebraically, so only the first
    TB[b] K tiles affect the output.  A host-side fp64 guard checks
    sigma = masked softmax mass > 1.5e-12 for every batch (the clamp
    fires at 1e-12; sigma's fp uncertainty is ~1e-5 relative, so the
    margin is decisive).  If any batch is too close, the kernel falls
    back to a full-K program that reproduces the reference exactly,
    including the clamp (denominator max(S_m, eps*Z)).
  - Since Q is non-increasing, the slots needing tile t are always a
    prefix: K and V load one prefix-packed tile [128p, a(t) slots, 512d]
    per t (contiguous 2-8 KiB runs).

Per-core dataflow:
  - q arrives pre-broadcast from the host ([128, 4*512], first on the ring).
  - energy via fused multiply+reduce (affine_mul_reduce) on DVE.
  - softmax: critical path to the context is only max (DVE reduce +
    gpsimd partition_all_reduce) -> exp (ACT, fused row sums) -> mask
    (DVE fused mask+mult+sum) -> PE transpose.  The normalizer
    r = 1/max(S_m, eps*Z) is computed off-path and folded into the final
    context copy (ACT scale) and the w output scale.
  - context: fp32 skinny matmuls over unnormalized weights, scheduled
    across the 4 PE column groups (tile_position) so up to 4 accumulation
    chains run concurrently; split-slot partials merged at the end.
  - w rows beyond the loaded K tiles stay zero via the runtime's
    zero-initialized output buffers, matching the reference mask.
"""
import contextlib
import os

import numpy as np

S, B, D = 2048, 32, 512
NCORES = 8
BL = B // NCORES          # 4 local batches (slots) per core
P = 128                   # partitions
NT = S // P               # 16 s-tiles of 128
EPS = 1e-12

_CACHE = {}
LAST_RESULT = None


def _chain_assignment(Q):
    """Assign each slot's context matmuls to PE column groups.  A slot
    larger than the balanced share is split across two groups; others get
    one group (no merge needed)."""
    total = sum(Q)
    share = (total + BL - 1) // BL
    loads = [0] * BL
    chains = {}          # slot -> list of (group, t_list)
    for s in sorted(range(BL), key=lambda x: -Q[x]):
        ts = list(range(Q[s]))
        if Q[s] > share and len(ts) >= 2:
            h = (Q[s] + 1) // 2
            parts = [ts[:h], ts[h:]]
        else:
            parts = [ts]
        lst = []
        for part in parts:
            g = min(range(BL), key=lambda x: loads[x])
            loads[g] += len(part)
            lst.append((g, part))
        chains[s] = lst
    return chains


def _build(Q, QK):
    from concourse import bacc, mybir
    import concourse.tile as tile
    from concourse import bass_isa

    F32 = mybir.dt.float32
    AX = mybir.AxisListType.X
    ALU = mybir.AluOpType
    EXP = mybir.ActivationFunctionType.Exp

    def nka(t):
        return sum(1 for s in range(BL) if t < QK[s])

    def nva(t):
        return sum(1 for s in range(BL) if t < Q[s])

    nc = bacc.Bacc("TRN2", target_bir_lowering=False, debug=False,
                   num_devices=NCORES)

    K_in = nc.dram_tensor("K_in", [S, BL, D], F32, kind="ExternalInput").ap()
    V_in = nc.dram_tensor("V_in", [S, BL, D], F32, kind="ExternalInput").ap()
    qb_in = nc.dram_tensor("qb_in", [P, BL * D], F32, kind="ExternalInput").ap()
    th_in = nc.dram_tensor("th_in", [NT, BL], F32, kind="ExternalInput").ap()
    io_in = nc.dram_tensor("io_in", [NT, P], F32, kind="ExternalInput").ap()
    id_in = nc.dram_tensor("id_in", [P, P], F32, kind="ExternalInput").ap()
    ctx_out = nc.dram_tensor("ctx_out", [1, BL * D], F32,
                             kind="ExternalOutput").ap()
    w_out = nc.dram_tensor("w_out", [BL, NT, P], F32, kind="ExternalOutput").ap()

    # prefix-packed tiles: tile t = [128p, a(t) slots, 512d]
    Kv = K_in.rearrange("(t p) b d -> t p b d", t=NT, p=P)
    Vv = V_in.rearrange("(t p) b d -> t p b d", t=NT, p=P)

    chains = _chain_assignment(Q)

    with tile.TileContext(nc) as tc:
        with contextlib.ExitStack() as ctx:
            kp = ctx.enter_context(tc.tile_pool(name="kp", bufs=1))
            vp = ctx.enter_context(tc.tile_pool(name="vp", bufs=1))
            cb = ctx.enter_context(tc.tile_pool(name="cb", bufs=1))
            tp = ctx.enter_context(tc.tile_pool(name="tp", bufs=2))
            wp = ctx.enter_context(tc.tile_pool(name="wp", bufs=4))
            pp = ctx.enter_context(tc.tile_pool(name="pp", bufs=1, space="PSUM"))

            # --- q (pre-broadcast) + constants first on the sync ring ---
            qbsb = cb.tile([P, BL * D], F32)
            nc.sync.dma_start(qbsb[:], qb_in[:])
            ones = cb.tile([NT, P], F32)
            nc.gpsimd.memset(ones[:], 1.0)

            # --- big-stream DMAs: ragged K then ragged V (all resident) ---
            ktiles, vtiles = [], []
            for t_i in range(NT):
                a = nka(t_i)
                if a == 0:
                    ktiles.append(None)
                    continue
                t = kp.tile([P, a, D], F32, tag=f"kt{t_i}", name=f"kt{t_i}",
                            bufs=1)
                nc.sync.dma_start(t[:], Kv[t_i][:, 0:a, :])
                ktiles.append(t)
            for t_i in range(NT):
                a = nva(t_i)
                if a == 0:
                    vtiles.append(None)
                    continue
                t = vp.tile([P, a, D], F32, tag=f"vt{t_i}", name=f"vt{t_i}",
                            bufs=1)
                nc.sync.dma_start(t[:], Vv[t_i][:, 0:a, :])
                vtiles.append(t)

            # softmax-phase constants (needed ~mid-kernel)
            ident = cb.tile([P, P], F32)
            nc.sync.dma_start(ident[:], id_in[:])
            iota = cb.tile([NT, P], F32)
            nc.sync.dma_start(iota[:], io_in[:])
            th = cb.tile([NT, BL], F32)
            nc.sync.dma_start(th[:], th_in[:])

            # --- energy: E_b[:, t] = sum_d K[s,:]*q (fused DVE), K-paced ---
            Es = [cb.tile([P, QK[b] + 1], F32, tag=f"E{b}", name=f"E{b}")
                  for b in range(BL)]
            for t in range(NT):
                for b in range(BL):
                    if t < QK[b]:
                        tmp = tp.tile([P, D], F32, tag="tmp")
                        nc.vector.affine_mul_reduce(
                            out=tmp[:], accum_out=Es[b][:, t:t + 1],
                            in0=ktiles[t][:, b, :],
                            in1=qbsb[:, b * D:(b + 1) * D],
                            scale=1.0, bias=0.0)

            # --- per-slot softmax (shapes sized to QK[b]) ---
            wTs, r1s = [], []
            for b in range(BL):
                E = Es[b]
                n = QK[b]
                # critical path: max -> exp -> mask -> transpose
                nc.vector.reduce_max(E[:, n:n + 1], E[:, 0:n], axis=AX)
                ar = wp.tile([P, 1], F32, tag="ar")
                nc.gpsimd.partition_all_reduce(
                    ar[:], E[:, n:n + 1], channels=P,
                    reduce_op=bass_isa.ReduceOp.max)
                nm = wp.tile([n, 1], F32, tag="nm", name="nm")
                nc.vector.tensor_scalar_mul(nm[:], ar[0:n, :], -1.0)

                Et_ps = pp.tile([n, P], F32, tag="Et_ps", name="Et_ps")
                nc.tensor.transpose(Et_ps[:], E[:, 0:n], ident[:])
                Et = wp.tile([n, P], F32, tag="Et", name="Et")
                nc.scalar.copy(Et[:], Et_ps[:])

                X = wp.tile([n, P], F32, tag="X", name="X")
                scat = wp.tile([n, 2], F32, tag="scat", name="scat")
                nc.scalar.activation(X[:], Et[:], EXP, bias=nm[:], scale=1.0,
                                     accum_out=scat[:, 0:1])

                W1 = wp.tile([n, P], F32, tag=f"W1{b}", name=f"W1{b}")
                nc.vector.scalar_tensor_tensor(
                    out=W1[:], in0=iota[0:n, :], scalar=th[0:n, b:b + 1],
                    in1=X[:], op0=ALU.is_lt, op1=ALU.mult,
                    accum_out=scat[:, 1:2])

                wT_ps = pp.tile([P, n], F32, tag="wT_ps", name="wT_ps")
                nc.tensor.transpose(wT_ps[:], W1[:], ident[0:n, 0:n])
                wT = wp.tile([P, n], F32, tag=f"wT{b}", name=f"wT{b}")
                nc.scalar.copy(wT[:], wT_ps[:])
                wTs.append(wT)

                # off path: normalizer r = 1/max(S_m, eps*Z), w output
                ss_ps = pp.tile([1, 2], F32, tag="sm_ps", name="ss_ps")
                nc.tensor.matmul(ss_ps[:], ones[0:n, 0:1], scat[:],
                                 start=True, stop=True)
                ss = wp.tile([1, 2], F32, tag="ss")
                nc.scalar.copy(ss[:], ss_ps[:])
                d1 = wp.tile([1, 1], F32, tag="d1")
                nc.vector.tensor_scalar_mul(d1[:], ss[:, 0:1], EPS)
                d2 = wp.tile([1, 1], F32, tag="d2")
                nc.vector.tensor_scalar_max(d2[:], ss[:, 1:2], d1[:])
                r1 = wp.tile([1, 1], F32, tag=f"r1{b}", name=f"r1{b}")
                nc.vector.reciprocal(r1[:], d2[:])
                r1s.append(r1)
                r_ps = pp.tile([n, 1], F32, tag="sm_ps", name="r_ps")
                nc.tensor.matmul(r_ps[:], ones[0:1, 0:n], r1[:],
                                 start=True, stop=True)
                r16 = wp.tile([n, 1], F32, tag="r16", name="r16")
                nc.scalar.copy(r16[:], r_ps[:])
                W = wp.tile([n, P], F32, tag=f"W{b}", name=f"W{b}")
                nc.vector.tensor_scalar_mul(W[:], W1[:], r16[:])
                nc.sync.dma_start(w_out[b, 0:n], W[:])

            # --- context: chains across PE column groups ---
            cps = [pp.tile([P, D], F32, tag="ctx_ps", bufs=BL, name=f"cps{b}")
                   for b in range(BL)]
            group_of, first_last = {}, {}
            for s in range(BL):
                for g, ts in chains[s]:
                    for t in ts:
                        group_of[(s, t)] = g
                    first_last[(s, g)] = (ts[0], ts[-1])
            for t in range(NT):
                for s in range(BL):
                    if t < Q[s]:
                        g = group_of[(s, t)]
                        ft, lt = first_last[(s, g)]
                        nc.tensor.matmul(
                            cps[s][32 * g:32 * g + 1, :],
                            wTs[s][:, t:t + 1], vtiles[t][:, s, :],
                            start=(t == ft), stop=(t == lt),
                            tile_position=(0, 32 * g))

            # merge partial chains, scale by r, single output DMA
            cs_row = cb.tile([1, BL * D], F32)
            for s in range(BL):
                gl = chains[s]
                dst = cs_row[0:1, s * D:(s + 1) * D]
                if len(gl) == 1:
                    g = gl[0][0]
                    nc.scalar.mul(dst, cps[s][32 * g:32 * g + 1, :], r1s[s][:])
                else:
                    gA, gB = gl[0][0], gl[1][0]
                    mgB = wp.tile([1, D], F32, tag=f"mgB{s}", name=f"mgB{s}")
                    nc.scalar.copy(mgB[:], cps[s][32 * gB:32 * gB + 1, :])
                    mg = wp.tile([1, D], F32, tag=f"mg{s}", name=f"mg{s}")
                    nc.vector.tensor_add(mg[:], cps[s][32 * gA:32 * gA + 1, :],
                                         mgB[:])
                    nc.scalar.mul(dst, mg[:], r1s[s][:])
            nc.sync.dma_start(ctx_out[:], cs_row[:])

    nc.compile()
    return nc


def _get_nc(Q, QK):
    key = (Q, QK)
    if key not in _CACHE:
        _CACHE[key] = _build(Q, QK)
    return _CACHE[key]


def _maybe_register_trace_hook():
    """run_bass_kernel_spmd(trace=True) under axon needs antenv.axon_hooks;
    the RL image lacks it, so synthesize the module from trn_agent_boot."""
    import sys, types
    if "antenv.axon_hooks" in sys.modules:
        return
    try:
        import trn_agent_boot.trn_boot as tb
        hook = tb._ntff_profile_via_ctypes('/opt/axon/libaxon_pjrt.so')
        mod = types.ModuleType('antenv.axon_hooks')
        mod.get_axon_ntff_profile_hook = lambda: hook
        mod.set_axon_ntff_profile_hook = lambda h: None
        sys.modules['antenv.axon_hooks'] = mod
    except Exception:
        pass


def _clamp_guard_ok(key, query, lens):
    """True iff no batch's masked softmax mass is close enough to the
    reference's 1e-12 clamp for the clamp decision to be in doubt."""
    e = np.matmul(key.transpose(1, 0, 2), query[:, :, None])[:, :, 0]  # (B,S)
    M = e.max(axis=1, keepdims=True)
    ex = np.exp((e - M).astype(np.float64))
    sigma = (ex * (np.arange(S)[None, :] < lens[:, None])).sum(axis=1) \
        / ex.sum(axis=1)
    return bool((sigma > 1.5e-12).all())


def kernel(key, value, query, encoder_condensed_lens):
    global LAST_RESULT
    key = np.asarray(key, dtype=np.float32)
    value = np.asarray(value, dtype=np.float32)
    query = np.asarray(query, dtype=np.float32)
    lens = np.asarray(encoder_condensed_lens).astype(np.int64).reshape(B)

    # rank-balanced assignment: sort batches by V tile count (desc); core c
    # slot k <- rank 8k+c.  Q[k] = max tile count within slot k = rank 8k.
    TB = np.clip(np.ceil(lens / P).astype(int), 1, NT)
    order = np.argsort(-TB, kind="stable")
    Q = tuple(int(TB[order[8 * k]]) for k in range(BL))
    QK = Q if _clamp_guard_ok(key, query, lens) else (NT,) * BL

    iota_np = np.tile(np.arange(P, dtype=np.float32)[None, :], (NT, 1))
    ident_np = np.eye(P, dtype=np.float32)

    in_maps = []
    batch_of = np.empty((NCORES, BL), dtype=int)
    for c in range(NCORES):
        bsel = [int(order[8 * k + c]) for k in range(BL)]
        batch_of[c] = bsel
        th = np.empty((NT, BL), dtype=np.float32)
        for k in range(BL):
            th[:, k] = lens[bsel[k]] - P * np.arange(NT)
        qb = np.tile(query[bsel, :].reshape(1, BL * D), (P, 1))
        in_maps.append({
            "K_in": np.ascontiguousarray(key[:, bsel, :]),
            "V_in": np.ascontiguousarray(value[:, bsel, :]),
            "qb_in": qb,
            "th_in": th,
            "io_in": iota_np,
            "id_in": ident_np,
        })

    nc = _get_nc(Q, QK)
    from concourse import bass_utils
    trace = bool(os.environ.get("BASS_TRACE"))
    if trace:
        _maybe_register_trace_hook()
    res = bass_utils.run_bass_kernel_spmd(
        nc, in_maps, core_ids=list(range(NCORES)), trace=trace)
    LAST_RESULT = res

    context = np.empty((B, D), dtype=np.float32)
    w = np.empty((B, S), dtype=np.float32)
    for c in range(NCORES):
        r = res.results[c]
        ctxc = r["ctx_out"].reshape(BL, D)
        for k in range(BL):
            context[batch_of[c][k]] = ctxc[k]
            w[batch_of[c][k]] = r["w_out"][k].reshape(S)
    return context, w


# revision 19
# speedup vs baseline: 1.2463x; 1.1872x over previous
"""Trainium2 Bass kernel for ragged-sequence attention.

reference computation (per batch b):
    energy[s] = sum_d key[s,b,:] . query[b,:]          (S=2048, B=32, D=512)
    w = softmax(energy) * mask;  w /= max(sum(w), eps)
    context = sum_s w[s] * value[s,b,:]
returns (context (B,D), w (B,S)).

Sharding: data parallel over batch, 4 batches per core on 8 cores, with a
rank-balanced batch->core assignment that equalizes ragged traffic:
batches sorted by tile count TB=ceil(len/128); core c gets ranks
{c, 8+c, 16+c, 24+c}; slot k's compiled tile count is Q[k] = TB[rank 8k].

Raggedness:
  - V: w is exactly zero for s >= len_b, so V rows beyond 128*TB[b] never
    contribute.  Only Q[k] V tiles are loaded per slot.
  - K: the reference normalizes by max(sum(w_masked), eps) with
    eps = 1e-12.  Whenever that clamp does NOT fire, the softmax max and
    the full-row normalizer cancel algebraically, so only the first
    TB[b] K tiles affect the output.  A host-side fp64 guard checks
    sigma = masked softmax mass > 1.5e-12 for every batch (the clamp
    fires at 1e-12; sigma's fp uncertainty is ~1e-5 relative, so the
    margin is decisive).  If any batch is too close, the kernel falls
    back to a full-K program that reproduces the reference exactly,
    including the clamp (denominator max(S_m, eps*Z)).
  - Since Q is non-increasing, the slots needing tile t are always a
    prefix: K and V load one prefix-packed tile [128p, a(t) slots, 512d]
    per t (contiguous 2-8 KiB runs).

Per-core dataflow:
  - q arrives pre-broadcast from the host ([128, 4*512], first on the ring).
  - energy via fused multiply+reduce (affine_mul_reduce) on DVE.
  - softmax: critical path to the context is only max (DVE reduce +
    gpsimd partition_all_reduce) -> exp (ACT, fused row sums) -> mask
    (DVE fused mask+mult+sum) -> PE transpose.  The normalizer
    r = 1/max(S_m, eps*Z) is computed off-path and folded into the final
    context copy (ACT scale) and the w output scale.
  - context: fp32 skinny matmuls over unnormalized weights, scheduled
    across the 4 PE column groups (tile_position) so up to 4 accumulation
    chains run concurrently; split-slot partials merged at the end.
  - w rows beyond the loaded K tiles stay zero via the runtime's
    zero-initialized output buffers, matching the reference mask.
"""
import contextlib
import os

import numpy as np

S, B, D = 2048, 32, 512
NCORES = 8
BL = B // NCORES          # 4 local batches (slots) per core
P = 128                   # partitions
NT = S // P               # 16 s-tiles of 128
EPS = 1e-12

_CACHE = {}
LAST_RESULT = None


def _chain_assignment(Q):
    """Assign each slot's context matmuls to PE column groups.  A slot
    larger than the balanced share is split across two groups; others get
    one group (no merge needed)."""
    total = sum(Q)
    share = (total + BL - 1) // BL
    loads = [0] * BL
    chains = {}          # slot -> list of (group, t_list)
    for s in sorted(range(BL), key=lambda x: -Q[x]):
        ts = list(range(Q[s]))
        if Q[s] > share and len(ts) >= 2:
            h = (Q[s] + 1) // 2
            parts = [ts[:h], ts[h:]]
        else:
            parts = [ts]
        lst = []
        for part in parts:
            g = min(range(BL), key=lambda x: loads[x])
            loads[g] += len(part)
            lst.append((g, part))
        chains[s] = lst
    return chains


def _build(Q, QK):
    from concourse import bacc, mybir
    import concourse.tile as tile
    from concourse import bass_isa

    F32 = mybir.dt.float32
    AX = mybir.AxisListType.X
    ALU = mybir.AluOpType
    EXP = mybir.ActivationFunctionType.Exp

    def nka(t):
        return sum(1 for s in range(BL) if t < QK[s])

    def nva(t):
        return sum(1 for s in range(BL) if t < Q[s])

    nc = bacc.Bacc("TRN2", target_bir_lowering=False, debug=False,
                   num_devices=NCORES)

    K_in = nc.dram_tensor("K_in", [S, BL, D], F32, kind="ExternalInput").ap()
    V_in = nc.dram_tensor("V_in", [S, BL, D], F32, kind="ExternalInput").ap()
    qb_in = nc.dram_tensor("qb_in", [P, BL * D], F32, kind="ExternalInput").ap()
    th_in = nc.dram_tensor("th_in", [NT, BL], F32, kind="ExternalInput").ap()
    io_in = nc.dram_tensor("io_in", [NT, P], F32, kind="ExternalInput").ap()
    id_in = nc.dram_tensor("id_in", [P, P], F32, kind="ExternalInput").ap()
    ctx_out = nc.dram_tensor("ctx_out", [1, BL * D], F32,
                             kind="ExternalOutput").ap()
    w_out = nc.dram_tensor("w_out", [BL, NT, P], F32, kind="ExternalOutput").ap()

    # prefix-packed tiles: tile t = [128p, a(t) slots, 512d]
    Kv = K_in.rearrange("(t p) b d -> t p b d", t=NT, p=P)
    Vv = V_in.rearrange("(t p) b d -> t p b d", t=NT, p=P)

    chains = _chain_assignment(Q)

    with tile.TileContext(nc) as tc:
        with contextlib.ExitStack() as ctx:
            kp = ctx.enter_context(tc.tile_pool(name="kp", bufs=1))
            vp = ctx.enter_context(tc.tile_pool(name="vp", bufs=1))
            cb = ctx.enter_context(tc.tile_pool(name="cb", bufs=1))
            tp = ctx.enter_context(tc.tile_pool(name="tp", bufs=2))
            wp = ctx.enter_context(tc.tile_pool(name="wp", bufs=4))
            pp = ctx.enter_context(tc.tile_pool(name="pp", bufs=1, space="PSUM"))

            # --- q (pre-broadcast) + constants first on the sync ring ---
            qbsb = cb.tile([P, BL * D], F32)
            nc.sync.dma_start(qbsb[:], qb_in[:])
            ones = cb.tile([NT, P], F32)
            nc.gpsimd.memset(ones[:], 1.0)

            # --- big-stream DMAs: ragged K then ragged V (all resident) ---
            ktiles, vtiles = [], []
            for t_i in range(NT):
                a = nka(t_i)
                if a == 0:
                    ktiles.append(None)
                    continue
                t = kp.tile([P, a, D], F32, tag=f"kt{t_i}", name=f"kt{t_i}",
                            bufs=1)
                nc.sync.dma_start(t[:], Kv[t_i][:, 0:a, :])
                ktiles.append(t)
            # softmax-phase constants: after K (not gating energy),
            # before V (must beat the softmax phase)
            ident = cb.tile([P, P], F32)
            nc.sync.dma_start(ident[:], id_in[:])
            iota = cb.tile([NT, P], F32)
            nc.sync.dma_start(iota[:], io_in[:])
            th = cb.tile([NT, BL], F32)
            nc.sync.dma_start(th[:], th_in[:])

            for t_i in range(NT):
                a = nva(t_i)
                if a == 0:
                    vtiles.append(None)
                    continue
                t = vp.tile([P, a, D], F32, tag=f"vt{t_i}", name=f"vt{t_i}",
                            bufs=1)
                nc.sync.dma_start(t[:], Vv[t_i][:, 0:a, :])
                vtiles.append(t)

            # --- energy: E_b[:, t] = sum_d K[s,:]*q (fused DVE), K-paced ---
            Es = [cb.tile([P, QK[b] + 1], F32, tag=f"E{b}", name=f"E{b}")
                  for b in range(BL)]
            for t in range(NT):
                for b in range(BL):
                    if t < QK[b]:
                        tmp = tp.tile([P, D], F32, tag="tmp")
                        nc.vector.affine_mul_reduce(
                            out=tmp[:], accum_out=Es[b][:, t:t + 1],
                            in0=ktiles[t][:, b, :],
                            in1=qbsb[:, b * D:(b + 1) * D],
                            scale=1.0, bias=0.0)

            # --- per-slot softmax (shapes sized to QK[b]) ---
            wTs, r1s = [], []
            for b in range(BL):
                E = Es[b]
                n = QK[b]
                # critical path: max -> exp -> mask -> transpose
                nc.vector.reduce_max(E[:, n:n + 1], E[:, 0:n], axis=AX)
                ar = wp.tile([P, 1], F32, tag="ar")
                nc.gpsimd.partition_all_reduce(
                    ar[:], E[:, n:n + 1], channels=P,
                    reduce_op=bass_isa.ReduceOp.max)
                nm = wp.tile([n, 1], F32, tag="nm", name="nm")
                nc.vector.tensor_scalar_mul(nm[:], ar[0:n, :], -1.0)

                Et_ps = pp.tile([n, P], F32, tag="Et_ps", name="Et_ps")
                nc.tensor.transpose(Et_ps[:], E[:, 0:n], ident[:])
                Et = wp.tile([n, P], F32, tag="Et", name="Et")
                nc.scalar.copy(Et[:], Et_ps[:])

                X = wp.tile([n, P], F32, tag="X", name="X")
                scat = wp.tile([n, 2], F32, tag="scat", name="scat")
                nc.scalar.activation(X[:], Et[:], EXP, bias=nm[:], scale=1.0,
                                     accum_out=scat[:, 0:1])

                W1 = wp.tile([n, P], F32, tag=f"W1{b}", name=f"W1{b}")
                nc.vector.scalar_tensor_tensor(
                    out=W1[:], in0=iota[0:n, :], scalar=th[0:n, b:b + 1],
                    in1=X[:], op0=ALU.is_lt, op1=ALU.mult,
                    accum_out=scat[:, 1:2])

                wT_ps = pp.tile([P, n], F32, tag="wT_ps", name="wT_ps")
                nc.tensor.transpose(wT_ps[:], W1[:], ident[0:n, 0:n])
                wT = wp.tile([P, n], F32, tag=f"wT{b}", name=f"wT{b}")
                nc.scalar.copy(wT[:], wT_ps[:])
                wTs.append(wT)

                # off path: normalizer r = 1/max(S_m, eps*Z), w output
                ss_ps = pp.tile([1, 2], F32, tag="sm_ps", name="ss_ps")
                nc.tensor.matmul(ss_ps[:], ones[0:n, 0:1], scat[:],
                                 start=True, stop=True)
                ss = wp.tile([1, 2], F32, tag="ss")
                nc.scalar.copy(ss[:], ss_ps[:])
                d1 = wp.tile([1, 1], F32, tag="d1")
                nc.vector.tensor_scalar_mul(d1[:], ss[:, 0:1], EPS)
                d2 = wp.tile([1, 1], F32, tag="d2")
                nc.vector.tensor_scalar_max(d2[:], ss[:, 1:2], d1[:])
                r1 = wp.tile([1, 1], F32, tag=f"r1{b}", name=f"r1{b}")
                nc.vector.reciprocal(r1[:], d2[:])
                r1s.append(r1)
                r_ps = pp.tile([n, 1], F32, tag="sm_ps", name="r_ps")
                nc.tensor.matmul(r_ps[:], ones[0:1, 0:n], r1[:],
                                 start=True, stop=True)
                r16 = wp.tile([n, 1], F32, tag="r16", name="r16")
                nc.scalar.copy(r16[:], r_ps[:])
                W = wp.tile([n, P], F32, tag=f"W{b}", name=f"W{b}")
                nc.vector.tensor_scalar_mul(W[:], W1[:], r16[:])
                nc.sync.dma_start(w_out[b, 0:n], W[:])

            # --- context: chains across PE column groups ---
            cps = [pp.tile([P, D], F32, tag="ctx_ps", bufs=BL, name=f"cps{b}")
                   for b in range(BL)]
            group_of, first_last = {}, {}
            for s in range(BL):
                for g, ts in chains[s]:
                    for t in ts:
                        group_of[(s, t)] = g
                    first_last[(s, g)] = (ts[0], ts[-1])
            for t in range(NT):
                for s in range(BL):
                    if t < Q[s]:
                        g = group_of[(s, t)]
                        ft, lt = first_last[(s, g)]
                        nc.tensor.matmul(
                            cps[s][32 * g:32 * g + 1, :],
                            wTs[s][:, t:t + 1], vtiles[t][:, s, :],
                            start=(t == ft), stop=(t == lt),
                            tile_position=(0, 32 * g))

            # merge partial chains, scale by r, single output DMA
            cs_row = cb.tile([1, BL * D], F32)
            for s in range(BL):
                gl = chains[s]
                dst = cs_row[0:1, s * D:(s + 1) * D]
                if len(gl) == 1:
                    g = gl[0][0]
                    nc.scalar.mul(dst, cps[s][32 * g:32 * g + 1, :], r1s[s][:])
                else:
                    gA, gB = gl[0][0], gl[1][0]
                    mgB = wp.tile([1, D], F32, tag=f"mgB{s}", name=f"mgB{s}")
                    nc.scalar.copy(mgB[:], cps[s][32 * gB:32 * gB + 1, :])
                    mg = wp.tile([1, D], F32, tag=f"mg{s}", name=f"mg{s}")
                    nc.vector.tensor_add(mg[:], cps[s][32 * gA:32 * gA + 1, :],
                                         mgB[:])
                    nc.scalar.mul(dst, mg[:], r1s[s][:])
            nc.sync.dma_start(ctx_out[:], cs_row[:])

    nc.compile()
    return nc


def _get_nc(Q, QK):
    key = (Q, QK)
    if key not in _CACHE:
        _CACHE[key] = _build(Q, QK)
    return _CACHE[key]


def _maybe_register_trace_hook():
    """run_bass_kernel_spmd(trace=True) under axon needs antenv.axon_hooks;
    the RL image lacks it, so synthesize the module from trn_agent_boot."""
    import sys, types
    if "antenv.axon_hooks" in sys.modules:
        return
    try:
        import trn_agent_boot.trn_boot as tb
        hook = tb._ntff_profile_via_ctypes('/opt/axon/libaxon_pjrt.so')
        mod = types.ModuleType('antenv.axon_hooks')
        mod.get_axon_ntff_profile_hook = lambda: hook
        mod.set_axon_ntff_profile_hook = lambda h: None
        sys.modules['antenv.axon_hooks'] = mod
    except Exception:
        pass


def _clamp_guard_ok(key, query, lens):
    """True iff no batch's masked softmax mass is close enough to the
    reference's 1e-12 clamp for the clamp decision to be in doubt."""
    e = np.matmul(key.transpose(1, 0, 2), query[:, :, None])[:, :, 0]  # (B,S)
    M = e.max(axis=1, keepdims=True)
    ex = np.exp((e - M).astype(np.float64))
    sigma = (ex * (np.arange(S)[None, :] < lens[:, None])).sum(axis=1) \
        / ex.sum(axis=1)
    return bool((sigma > 1.5e-12).all())


def kernel(key, value, query, encoder_condensed_lens):
    global LAST_RESULT
    key = np.asarray(key, dtype=np.float32)
    value = np.asarray(value, dtype=np.float32)
    query = np.asarray(query, dtype=np.float32)
    lens = np.asarray(encoder_condensed_lens).astype(np.int64).reshape(B)

    # rank-balanced assignment: sort batches by V tile count (desc); core c
    # slot k <- rank 8k+c.  Q[k] = max tile count within slot k = rank 8k.
    TB = np.clip(np.ceil(lens / P).astype(int), 1, NT)
    order = np.argsort(-TB, kind="stable")
    Q = tuple(int(TB[order[8 * k]]) for k in range(BL))
    QK = Q if _clamp_guard_ok(key, query, lens) else (NT,) * BL

    iota_np = np.tile(np.arange(P, dtype=np.float32)[None, :], (NT, 1))
    ident_np = np.eye(P, dtype=np.float32)

    in_maps = []
    batch_of = np.empty((NCORES, BL), dtype=int)
    for c in range(NCORES):
        bsel = [int(order[8 * k + c]) for k in range(BL)]
        batch_of[c] = bsel
        th = np.empty((NT, BL), dtype=np.float32)
        for k in range(BL):
            th[:, k] = lens[bsel[k]] - P * np.arange(NT)
        qb = np.tile(query[bsel, :].reshape(1, BL * D), (P, 1))
        in_maps.append({
            "K_in": np.ascontiguousarray(key[:, bsel, :]),
            "V_in": np.ascontiguousarray(value[:, bsel, :]),
            "qb_in": qb,
            "th_in": th,
            "io_in": iota_np,
            "id_in": ident_np,
        })

    nc = _get_nc(Q, QK)
    from concourse import bass_utils
    trace = bool(os.environ.get("BASS_TRACE"))
    if trace:
        _maybe_register_trace_hook()
    res = bass_utils.run_bass_kernel_spmd(
        nc, in_maps, core_ids=list(range(NCORES)), trace=trace)
    LAST_RESULT = res

    context = np.empty((B, D), dtype=np.float32)
    w = np.empty((B, S), dtype=np.float32)
    for c in range(NCORES):
        r = res.results[c]
        ctxc = r["ctx_out"].reshape(BL, D)
        for k in range(BL):
            context[batch_of[c][k]] = ctxc[k]
            w[batch_of[c][k]] = r["w_out"][k].reshape(S)
    return context, w


# revision 20
# speedup vs baseline: 1.2638x; 1.0140x over previous
"""Trainium2 Bass kernel for ragged-sequence attention.

reference computation (per batch b):
    energy[s] = sum_d key[s,b,:] . query[b,:]          (S=2048, B=32, D=512)
    w = softmax(energy) * mask;  w /= max(sum(w), eps)
    context = sum_s w[s] * value[s,b,:]
returns (context (B,D), w (B,S)).

Sharding: data parallel over batch, 4 batches per core on 8 cores, with a
rank-balanced batch->core assignment that equalizes ragged traffic:
batches sorted by tile count TB=ceil(len/128); core c gets ranks
{c, 8+c, 16+c, 24+c}; slot k's compiled tile count is Q[k] = TB[rank 8k].

Raggedness:
  - V: w is exactly zero for s >= len_b, so V rows beyond 128*TB[b] never
    contribute.  Only Q[k] V tiles are loaded per slot.
  - K: the reference normalizes by max(sum(w_masked), eps) with
    eps = 1e-12.  Whenever that clamp does NOT fire, the softmax max and
    the full-row normalizer cancel algebraically, so only the first
    TB[b] K tiles affect the output.  A host-side fp64 guard checks
    sigma = masked softmax mass > 1.5e-12 for every batch (the clamp
    fires at 1e-12; sigma's fp uncertainty is ~1e-5 relative, so the
    margin is decisive).  If any batch is too close, the kernel falls
    back to a full-K program that reproduces the reference exactly,
    including the clamp (denominator max(S_m, eps*Z)).
  - Since Q is non-increasing, the slots needing tile t are always a
    prefix: K and V load one prefix-packed tile [128p, a(t) slots, 512d]
    per t (contiguous 2-8 KiB runs).

Per-core dataflow:
  - q arrives pre-broadcast from the host ([128, 4*512], first on the ring).
  - energy via fused multiply+reduce (affine_mul_reduce) on DVE.
  - softmax: critical path to the context is only max (DVE reduce +
    gpsimd partition_all_reduce) -> exp (ACT, fused row sums) -> mask
    (DVE fused mask+mult+sum) -> PE transpose.  The normalizer
    r = 1/max(S_m, eps*Z) is computed off-path and folded into the final
    context copy (ACT scale) and the w output scale.
  - context: fp32 skinny matmuls over unnormalized weights, scheduled
    across the 4 PE column groups (tile_position) so up to 4 accumulation
    chains run concurrently; split-slot partials merged at the end.
  - w rows beyond the loaded K tiles stay zero via the runtime's
    zero-initialized output buffers, matching the reference mask.
"""
import contextlib
import os

import numpy as np

S, B, D = 2048, 32, 512
NCORES = 8
BL = B // NCORES          # 4 local batches (slots) per core
P = 128                   # partitions
NT = S // P               # 16 s-tiles of 128
EPS = 1e-12

_CACHE = {}
LAST_RESULT = None


def _chain_assignment(Q):
    """Assign each slot's context matmuls to PE column groups.  A slot
    larger than the balanced share is split across two groups; others get
    one group (no merge needed)."""
    total = sum(Q)
    share = (total + BL - 1) // BL
    loads = [0] * BL
    chains = {}          # slot -> list of (group, t_list)
    for s in sorted(range(BL), key=lambda x: -Q[x]):
        ts = list(range(Q[s]))
        if Q[s] > share and len(ts) >= 2:
            h = (Q[s] + 1) // 2
            parts = [ts[:h], ts[h:]]
        else:
            parts = [ts]
        lst = []
        for part in parts:
            g = min(range(BL), key=lambda x: loads[x])
            loads[g] += len(part)
            lst.append((g, part))
        chains[s] = lst
    return chains


def _build(Q, QK):
    from concourse import bacc, mybir
    import concourse.tile as tile
    from concourse import bass_isa

    F32 = mybir.dt.float32
    AX = mybir.AxisListType.X
    ALU = mybir.AluOpType
    EXP = mybir.ActivationFunctionType.Exp

    def nka(t):
        return sum(1 for s in range(BL) if t < QK[s])

    def nva(t):
        return sum(1 for s in range(BL) if t < Q[s])

    nc = bacc.Bacc("TRN2", target_bir_lowering=False, debug=False,
                   num_devices=NCORES)

    K_in = nc.dram_tensor("K_in", [S, BL, D], F32, kind="ExternalInput").ap()
    V_in = nc.dram_tensor("V_in", [S, BL, D], F32, kind="ExternalInput").ap()
    qb_in = nc.dram_tensor("qb_in", [P, BL * D], F32, kind="ExternalInput").ap()
    th_in = nc.dram_tensor("th_in", [NT, BL], F32, kind="ExternalInput").ap()
    io_in = nc.dram_tensor("io_in", [NT, P], F32, kind="ExternalInput").ap()
    id_in = nc.dram_tensor("id_in", [P, P], F32, kind="ExternalInput").ap()
    ctx_out = nc.dram_tensor("ctx_out", [1, BL * D], F32,
                             kind="ExternalOutput").ap()
    w_out = nc.dram_tensor("w_out", [BL, NT, P], F32, kind="ExternalOutput").ap()

    # prefix-packed tiles: tile t = [128p, a(t) slots, 512d]
    Kv = K_in.rearrange("(t p) b d -> t p b d", t=NT, p=P)
    Vv = V_in.rearrange("(t p) b d -> t p b d", t=NT, p=P)

    chains = _chain_assignment(Q)

    with tile.TileContext(nc) as tc:
        with contextlib.ExitStack() as ctx:
            kp = ctx.enter_context(tc.tile_pool(name="kp", bufs=1))
            vp = ctx.enter_context(tc.tile_pool(name="vp", bufs=1))
            cb = ctx.enter_context(tc.tile_pool(name="cb", bufs=1))
            tp = ctx.enter_context(tc.tile_pool(name="tp", bufs=2))
            wp = ctx.enter_context(tc.tile_pool(name="wp", bufs=4))
            pp = ctx.enter_context(tc.tile_pool(name="pp", bufs=1, space="PSUM"))

            # --- q (pre-broadcast) + constants first on the sync ring ---
            qbsb = cb.tile([P, BL * D], F32)
            nc.sync.dma_start(qbsb[:], qb_in[:])
            ones = cb.tile([NT, P], F32)
            nc.gpsimd.memset(ones[:], 1.0)

            # --- big-stream DMAs: ragged K then ragged V.  Tiles stay
            # resident when they fit; otherwise same-width tiles rotate
            # through half as many buffers (consumption is stream-ordered).
            def width_bufs(afn):
                cnt = {}
                for x in range(NT):
                    if afn(x):
                        cnt[afn(x)] = cnt.get(afn(x), 0) + 1
                total_kb = sum(a * n * 256 for a, n in cnt.items())
                if total_kb <= 9 * 1024:
                    return cnt
                return {a: max(2, n // 2) for a, n in cnt.items()}

            kbufs, vbufs = width_bufs(nka), width_bufs(nva)
            ktiles, vtiles = [], []
            for t_i in range(NT):
                a = nka(t_i)
                if a == 0:
                    ktiles.append(None)
                    continue
                t = kp.tile([P, a, D], F32, tag=f"kt{a}", name=f"kt{a}",
                            bufs=kbufs[a])
                nc.sync.dma_start(t[:], Kv[t_i][:, 0:a, :])
                ktiles.append(t)
            # softmax-phase constants: after K (not gating energy),
            # before V (must beat the softmax phase)
            ident = cb.tile([P, P], F32)
            nc.sync.dma_start(ident[:], id_in[:])
            iota = cb.tile([NT, P], F32)
            nc.sync.dma_start(iota[:], io_in[:])
            th = cb.tile([NT, BL], F32)
            nc.sync.dma_start(th[:], th_in[:])

            for t_i in range(NT):
                a = nva(t_i)
                if a == 0:
                    vtiles.append(None)
                    continue
                t = vp.tile([P, a, D], F32, tag=f"vt{a}", name=f"vt{a}",
                            bufs=vbufs[a])
                nc.sync.dma_start(t[:], Vv[t_i][:, 0:a, :])
                vtiles.append(t)

            # --- energy: E_b[:, t] = sum_d K[s,:]*q (fused DVE), K-paced ---
            Es = [cb.tile([P, QK[b] + 1], F32, tag=f"E{b}", name=f"E{b}")
                  for b in range(BL)]
            for t in range(NT):
                for b in range(BL):
                    if t < QK[b]:
                        tmp = tp.tile([P, D], F32, tag="tmp")
                        nc.vector.affine_mul_reduce(
                            out=tmp[:], accum_out=Es[b][:, t:t + 1],
                            in0=ktiles[t][:, b, :],
                            in1=qbsb[:, b * D:(b + 1) * D],
                            scale=1.0, bias=0.0)

            # --- per-slot softmax (shapes sized to QK[b]) ---
            wTs, r1s = [], []
            for b in range(BL):
                E = Es[b]
                n = QK[b]
                # critical path: max -> exp -> mask -> transpose
                nc.vector.reduce_max(E[:, n:n + 1], E[:, 0:n], axis=AX)
                ar = wp.tile([P, 1], F32, tag="ar")
                nc.gpsimd.partition_all_reduce(
                    ar[:], E[:, n:n + 1], channels=P,
                    reduce_op=bass_isa.ReduceOp.max)
                nm = wp.tile([n, 1], F32, tag="nm", name="nm")
                nc.vector.tensor_scalar_mul(nm[:], ar[0:n, :], -1.0)

                Et_ps = pp.tile([n, P], F32, tag="Et_ps", name="Et_ps")
                nc.tensor.transpose(Et_ps[:], E[:, 0:n], ident[:])
                Et = wp.tile([n, P], F32, tag="Et", name="Et")
                nc.scalar.copy(Et[:], Et_ps[:])

                X = wp.tile([n, P], F32, tag="X", name="X")
                scat = wp.tile([n, 2], F32, tag="scat", name="scat")
                nc.scalar.activation(X[:], Et[:], EXP, bias=nm[:], scale=1.0,
                                     accum_out=scat[:, 0:1])

                W1 = wp.tile([n, P], F32, tag=f"W1{b}", name=f"W1{b}")
                nc.vector.scalar_tensor_tensor(
                    out=W1[:], in0=iota[0:n, :], scalar=th[0:n, b:b + 1],
                    in1=X[:], op0=ALU.is_lt, op1=ALU.mult,
                    accum_out=scat[:, 1:2])

                wT_ps = pp.tile([P, n], F32, tag="wT_ps", name="wT_ps")
                nc.tensor.transpose(wT_ps[:], W1[:], ident[0:n, 0:n])
                wT = wp.tile([P, n], F32, tag=f"wT{b}", name=f"wT{b}")
                nc.scalar.copy(wT[:], wT_ps[:])
                wTs.append(wT)

                # off path: normalizer r = 1/max(S_m, eps*Z), w output
                ss_ps = pp.tile([1, 2], F32, tag="sm_ps", name="ss_ps")
                nc.tensor.matmul(ss_ps[:], ones[0:n, 0:1], scat[:],
                                 start=True, stop=True)
                ss = wp.tile([1, 2], F32, tag="ss")
                nc.scalar.copy(ss[:], ss_ps[:])
                d1 = wp.tile([1, 1], F32, tag="d1")
                nc.vector.tensor_scalar_mul(d1[:], ss[:, 0:1], EPS)
                d2 = wp.tile([1, 1], F32, tag="d2")
                nc.vector.tensor_scalar_max(d2[:], ss[:, 1:2], d1[:])
                r1 = wp.tile([1, 1], F32, tag=f"r1{b}", name=f"r1{b}")
                nc.vector.reciprocal(r1[:], d2[:])
                r1s.append(r1)
                r_ps = pp.tile([n, 1], F32, tag="sm_ps", name="r_ps")
                nc.tensor.matmul(r_ps[:], ones[0:1, 0:n], r1[:],
                                 start=True, stop=True)
                r16 = wp.tile([n, 1], F32, tag="r16", name="r16")
                nc.scalar.copy(r16[:], r_ps[:])
                W = wp.tile([n, P], F32, tag=f"W{b}", name=f"W{b}")
                nc.vector.tensor_scalar_mul(W[:], W1[:], r16[:])
                nc.sync.dma_start(w_out[b, 0:n], W[:])

            # --- context: chains across PE column groups ---
            cps = [pp.tile([P, D], F32, tag="ctx_ps", bufs=BL, name=f"cps{b}")
                   for b in range(BL)]
            group_of, first_last = {}, {}
            for s in range(BL):
                for g, ts in chains[s]:
                    for t in ts:
                        group_of[(s, t)] = g
                    first_last[(s, g)] = (ts[0], ts[-1])
            for t in range(NT):
                for s in range(BL):
                    if t < Q[s]:
                        g = group_of[(s, t)]
                        ft, lt = first_last[(s, g)]
                        nc.tensor.matmul(
                            cps[s][32 * g:32 * g + 1, :],
                            wTs[s][:, t:t + 1], vtiles[t][:, s, :],
                            start=(t == ft), stop=(t == lt),
                            tile_position=(0, 32 * g))

            # merge partial chains, scale by r, single output DMA
            cs_row = cb.tile([1, BL * D], F32)
            for s in range(BL):
                gl = chains[s]
                dst = cs_row[0:1, s * D:(s + 1) * D]
                if len(gl) == 1:
                    g = gl[0][0]
                    nc.scalar.mul(dst, cps[s][32 * g:32 * g + 1, :], r1s[s][:])
                else:
                    gA, gB = gl[0][0], gl[1][0]
                    mgB = wp.tile([1, D], F32, tag=f"mgB{s}", name=f"mgB{s}")
                    nc.scalar.copy(mgB[:], cps[s][32 * gB:32 * gB + 1, :])
                    mg = wp.tile([1, D], F32, tag=f"mg{s}", name=f"mg{s}")
                    nc.vector.tensor_add(mg[:], cps[s][32 * gA:32 * gA + 1, :],
                                         mgB[:])
                    nc.scalar.mul(dst, mg[:], r1s[s][:])
            nc.sync.dma_start(ctx_out[:], cs_row[:])

    nc.compile()
    return nc


def _get_nc(Q, QK):
    key = (Q, QK)
    if key not in _CACHE:
        _CACHE[key] = _build(Q, QK)
    return _CACHE[key]


def _maybe_register_trace_hook():
    """run_bass_kernel_spmd(trace=True) under axon needs antenv.axon_hooks;
    the RL image lacks it, so synthesize the module from trn_agent_boot."""
    import sys, types
    if "antenv.axon_hooks" in sys.modules:
        return
    try:
        import trn_agent_boot.trn_boot as tb
        hook = tb._ntff_profile_via_ctypes('/opt/axon/libaxon_pjrt.so')
        mod = types.ModuleType('antenv.axon_hooks')
        mod.get_axon_ntff_profile_hook = lambda: hook
        mod.set_axon_ntff_profile_hook = lambda h: None
        sys.modules['antenv.axon_hooks'] = mod
    except Exception:
        pass


def _clamp_guard_ok(key, query, lens):
    """True iff no batch's masked softmax mass is close enough to the
    reference's 1e-12 clamp for the clamp decision to be in doubt."""
    e = np.matmul(key.transpose(1, 0, 2), query[:, :, None])[:, :, 0]  # (B,S)
    M = e.max(axis=1, keepdims=True)
    ex = np.exp((e - M).astype(np.float64))
    sigma = (ex * (np.arange(S)[None, :] < lens[:, None])).sum(axis=1) \
        / ex.sum(axis=1)
    return bool((sigma > 1.5e-12).all())


def kernel(key, value, query, encoder_condensed_lens):
    global LAST_RESULT
    key = np.asarray(key, dtype=np.float32)
    value = np.asarray(value, dtype=np.float32)
    query = np.asarray(query, dtype=np.float32)
    lens = np.asarray(encoder_condensed_lens).astype(np.int64).reshape(B)

    # rank-balanced assignment: sort batches by V tile count (desc); core c
    # slot k <- rank 8k+c.  Q[k] = max tile count within slot k = rank 8k.
    TB = np.clip(np.ceil(lens / P).astype(int), 1, NT)
    order = np.argsort(-TB, kind="stable")
    Q = tuple(int(TB[order[8 * k]]) for k in range(BL))
    QK = Q if _clamp_guard_ok(key, query, lens) else (NT,) * BL

    iota_np = np.tile(np.arange(P, dtype=np.float32)[None, :], (NT, 1))
    ident_np = np.eye(P, dtype=np.float32)

    in_maps = []
    batch_of = np.empty((NCORES, BL), dtype=int)
    for c in range(NCORES):
        bsel = [int(order[8 * k + c]) for k in range(BL)]
        batch_of[c] = bsel
        th = np.empty((NT, BL), dtype=np.float32)
        for k in range(BL):
            th[:, k] = lens[bsel[k]] - P * np.arange(NT)
        qb = np.tile(query[bsel, :].reshape(1, BL * D), (P, 1))
        in_maps.append({
            "K_in": np.ascontiguousarray(key[:, bsel, :]),
            "V_in": np.ascontiguousarray(value[:, bsel, :]),
            "qb_in": qb,
            "th_in": th,
            "io_in": iota_np,
            "id_in": ident_np,
        })

    nc = _get_nc(Q, QK)
    from concourse import bass_utils
    trace = bool(os.environ.get("BASS_TRACE"))
    if trace:
        _maybe_register_trace_hook()
    res = bass_utils.run_bass_kernel_spmd(
        nc, in_maps, core_ids=list(range(NCORES)), trace=trace)
    LAST_RESULT = res

    context = np.empty((B, D), dtype=np.float32)
    w = np.empty((B, S), dtype=np.float32)
    for c in range(NCORES):
        r = res.results[c]
        ctxc = r["ctx_out"].reshape(BL, D)
        for k in range(BL):
            context[batch_of[c][k]] = ctxc[k]
            w[batch_of[c][k]] = r["w_out"][k].reshape(S)
    return context, w


# revision 21
# speedup vs baseline: 1.4985x; 1.1857x over previous
"""Trainium2 Bass kernel for ragged-sequence attention.

reference computation (per batch b):
    energy[s] = sum_d key[s,b,:] . query[b,:]          (S=2048, B=32, D=512)
    w = softmax(energy) * mask;  w /= max(sum(w), eps)
    context = sum_s w[s] * value[s,b,:]
returns (context (B,D), w (B,S)).

Sharding: data parallel over batch, 4 batches per core on 8 cores, with a
rank-balanced batch->core assignment that equalizes ragged traffic:
batches sorted by tile count TB=ceil(len/128); core c gets ranks
{c, 8+c, 16+c, 24+c}; slot k's compiled tile count is Q[k] = TB[rank 8k].

Raggedness:
  - V: w is exactly zero for s >= len_b, so V rows beyond 128*TB[b] never
    contribute.  Only Q[k] V tiles are loaded per slot.
  - K: the reference normalizes by max(sum(w_masked), eps) with
    eps = 1e-12.  Whenever that clamp does NOT fire, the softmax max and
    the full-row normalizer cancel algebraically, so only the first
    TB[b] K tiles affect the output.  A host-side fp64 guard checks
    sigma = masked softmax mass > 1.5e-12 for every batch (the clamp
    fires at 1e-12; sigma's fp uncertainty is ~1e-5 relative, so the
    margin is decisive).  If any batch is too close, the kernel falls
    back to a full-K program that reproduces the reference exactly,
    including the clamp (denominator max(S_m, eps*Z)).
  - Since Q is non-increasing, the slots needing tile t are always a
    prefix: K and V load one prefix-packed tile [128p, a(t) slots, 512d]
    per t (contiguous 2-8 KiB runs).

Per-core dataflow:
  - q arrives pre-broadcast from the host ([128, 4*512], first on the ring).
  - energy via fused multiply+reduce (affine_mul_reduce) on DVE.
  - softmax: critical path to the context is only max (DVE reduce +
    gpsimd partition_all_reduce) -> exp (ACT, fused row sums) -> mask
    (DVE fused mask+mult+sum) -> PE transpose.  The normalizer
    r = 1/max(S_m, eps*Z) is computed off-path and folded into the final
    context copy (ACT scale) and the w output scale.
  - context: fp32 skinny matmuls over unnormalized weights, scheduled
    across the 4 PE column groups (tile_position) so up to 4 accumulation
    chains run concurrently; split-slot partials merged at the end.
  - w rows beyond the loaded K tiles stay zero via the runtime's
    zero-initialized output buffers, matching the reference mask.
"""
import contextlib
import os

import numpy as np

S, B, D = 2048, 32, 512
NCORES = 8
BL = B // NCORES          # 4 local batches (slots) per core
P = 128                   # partitions
NT = S // P               # 16 s-tiles of 128
EPS = 1e-12

_CACHE = {}
LAST_RESULT = None


def _chain_assignment(Q):
    """Assign each slot's context matmuls to PE column groups.  A slot
    larger than the balanced share is split across two groups; others get
    one group (no merge needed)."""
    total = sum(Q)
    share = (total + BL - 1) // BL
    loads = [0] * BL
    chains = {}          # slot -> list of (group, t_list)
    for s in sorted(range(BL), key=lambda x: -Q[x]):
        ts = list(range(Q[s]))
        if Q[s] > share and len(ts) >= 2:
            h = (Q[s] + 1) // 2
            parts = [ts[:h], ts[h:]]
        else:
            parts = [ts]
        lst = []
        for part in parts:
            g = min(range(BL), key=lambda x: loads[x])
            loads[g] += len(part)
            lst.append((g, part))
        chains[s] = lst
    return chains


def _build(Q, QK):
    from concourse import bacc, mybir
    import concourse.tile as tile
    from concourse import bass_isa

    F32 = mybir.dt.float32
    AX = mybir.AxisListType.X
    ALU = mybir.AluOpType
    EXP = mybir.ActivationFunctionType.Exp

    def nka(t):
        return sum(1 for s in range(BL) if t < QK[s])

    def nva(t):
        return sum(1 for s in range(BL) if t < Q[s])

    nc = bacc.Bacc("TRN2", target_bir_lowering=False, debug=False,
                   num_devices=NCORES)

    K_in = nc.dram_tensor("K_in", [S, BL, D], F32, kind="ExternalInput").ap()
    V_in = nc.dram_tensor("V_in", [S, BL, D], F32, kind="ExternalInput").ap()
    qb_in = nc.dram_tensor("qb_in", [P, BL * D], F32, kind="ExternalInput").ap()
    th_in = nc.dram_tensor("th_in", [NT, BL], F32, kind="ExternalInput").ap()
    io_in = nc.dram_tensor("io_in", [NT, P], F32, kind="ExternalInput").ap()
    id_in = nc.dram_tensor("id_in", [P, P], F32, kind="ExternalInput").ap()
    ctx_out = nc.dram_tensor("ctx_out", [1, BL * D], F32,
                             kind="ExternalOutput").ap()
    w_out = nc.dram_tensor("w_out", [BL, NT, P], F32, kind="ExternalOutput").ap()

    # prefix-packed tiles: tile t = [128p, a(t) slots, 512d]
    Kv = K_in.rearrange("(t p) b d -> t p b d", t=NT, p=P)
    Vv = V_in.rearrange("(t p) b d -> t p b d", t=NT, p=P)

    chains = _chain_assignment(Q)

    with tile.TileContext(nc) as tc:
        with contextlib.ExitStack() as ctx:
            kp = ctx.enter_context(tc.tile_pool(name="kp", bufs=1))
            vp = ctx.enter_context(tc.tile_pool(name="vp", bufs=1))
            cb = ctx.enter_context(tc.tile_pool(name="cb", bufs=1))
            tp = ctx.enter_context(tc.tile_pool(name="tp", bufs=2))
            wp = ctx.enter_context(tc.tile_pool(name="wp", bufs=4))
            pp = ctx.enter_context(tc.tile_pool(name="pp", bufs=1, space="PSUM"))

            # --- q (pre-broadcast) + constants first on the sync ring ---
            qbsb = cb.tile([P, BL * D], F32)
            nc.sync.dma_start(qbsb[:], qb_in[:])
            ones = cb.tile([NT, P], F32)
            nc.gpsimd.memset(ones[:], 1.0)

            # --- big-stream DMAs: ragged K then ragged V.  Tiles stay
            # resident when they fit; otherwise same-width tiles rotate
            # through half as many buffers (consumption is stream-ordered).
            def width_bufs(afn):
                cnt = {}
                for x in range(NT):
                    if afn(x):
                        cnt[afn(x)] = cnt.get(afn(x), 0) + 1
                total_kb = sum(a * n * 256 for a, n in cnt.items())
                if total_kb <= 9 * 1024:
                    return cnt
                return {a: max(2, n // 2) for a, n in cnt.items()}

            kbufs, vbufs = width_bufs(nka), width_bufs(nva)
            ktiles, vtiles = [], []
            for t_i in range(NT):
                a = nka(t_i)
                if a == 0:
                    ktiles.append(None)
                    continue
                t = kp.tile([P, a, D], F32, tag=f"kt{a}", name=f"kt{a}",
                            bufs=kbufs[a])
                nc.sync.dma_start(t[:], Kv[t_i][:, 0:a, :])
                ktiles.append(t)
            # softmax-phase constants: after K (not gating energy),
            # before V (must beat the softmax phase)
            ident = cb.tile([P, P], F32)
            nc.sync.dma_start(ident[:], id_in[:])
            iota = cb.tile([NT, P], F32)
            nc.sync.dma_start(iota[:], io_in[:])
            th = cb.tile([NT, BL], F32)
            nc.sync.dma_start(th[:], th_in[:])

            for t_i in range(NT):
                a = nva(t_i)
                if a == 0:
                    vtiles.append(None)
                    continue
                t = vp.tile([P, a, D], F32, tag=f"vt{a}", name=f"vt{a}",
                            bufs=vbufs[a])
                nc.sync.dma_start(t[:], Vv[t_i][:, 0:a, :])
                vtiles.append(t)

            # --- energy: E_b[:, t] = sum_d K[s,:]*q (fused DVE), K-paced ---
            Es = [cb.tile([P, QK[b] + 1], F32, tag=f"E{b}", name=f"E{b}")
                  for b in range(BL)]
            for t in range(NT):
                for b in range(BL):
                    if t < QK[b]:
                        tmp = tp.tile([P, D], F32, tag="tmp")
                        nc.vector.affine_mul_reduce(
                            out=tmp[:], accum_out=Es[b][:, t:t + 1],
                            in0=ktiles[t][:, b, :],
                            in1=qbsb[:, b * D:(b + 1) * D],
                            scale=1.0, bias=0.0)

            # --- per-slot softmax (shapes sized to QK[b]) ---
            wTs, r1s = [], []
            for b in range(BL):
                E = Es[b]
                n = QK[b]
                # critical path: max -> exp -> mask -> transpose
                nc.vector.reduce_max(E[:, n:n + 1], E[:, 0:n], axis=AX)
                ar = wp.tile([P, 1], F32, tag="ar")
                nc.gpsimd.partition_all_reduce(
                    ar[:], E[:, n:n + 1], channels=P,
                    reduce_op=bass_isa.ReduceOp.max)
                nm = wp.tile([n, 1], F32, tag="nm", name="nm")
                nc.vector.tensor_scalar_mul(nm[:], ar[0:n, :], -1.0)

                Et_ps = pp.tile([n, P], F32, tag="Et_ps", name="Et_ps")
                nc.tensor.transpose(Et_ps[:], E[:, 0:n], ident[:])
                Et = wp.tile([n, P], F32, tag="Et", name="Et")
                nc.scalar.copy(Et[:], Et_ps[:])

                X = wp.tile([n, P], F32, tag="X", name="X")
                scat = wp.tile([n, 2], F32, tag="scat", name="scat")
                nc.scalar.activation(X[:], Et[:], EXP, bias=nm[:], scale=1.0,
                                     accum_out=scat[:, 0:1])

                W1 = wp.tile([n, P], F32, tag=f"W1{b}", name=f"W1{b}")
                nc.vector.scalar_tensor_tensor(
                    out=W1[:], in0=iota[0:n, :], scalar=th[0:n, b:b + 1],
                    in1=X[:], op0=ALU.is_lt, op1=ALU.mult,
                    accum_out=scat[:, 1:2])

                wT_ps = pp.tile([P, n], F32, tag="wT_ps", name="wT_ps")
                nc.tensor.transpose(wT_ps[:], W1[:], ident[0:n, 0:n])
                wT = wp.tile([P, n], F32, tag=f"wT{b}", name=f"wT{b}")
                nc.scalar.copy(wT[:], wT_ps[:])
                wTs.append(wT)

                # off path: normalizer r = 1/max(S_m, eps*Z), w output
                ss_ps = pp.tile([1, 2], F32, tag="sm_ps", name="ss_ps")
                nc.tensor.matmul(ss_ps[:], ones[0:n, 0:1], scat[:],
                                 start=True, stop=True)
                ss = wp.tile([1, 2], F32, tag="ss")
                nc.scalar.copy(ss[:], ss_ps[:])
                d1 = wp.tile([1, 1], F32, tag="d1")
                nc.vector.tensor_scalar_mul(d1[:], ss[:, 0:1], EPS)
                d2 = wp.tile([1, 1], F32, tag="d2")
                nc.vector.tensor_scalar_max(d2[:], ss[:, 1:2], d1[:])
                r1 = wp.tile([1, 1], F32, tag=f"r1{b}", name=f"r1{b}")
                nc.vector.reciprocal(r1[:], d2[:])
                r1s.append(r1)
                r_ps = pp.tile([n, 1], F32, tag="sm_ps", name="r_ps")
                nc.tensor.matmul(r_ps[:], ones[0:1, 0:n], r1[:],
                                 start=True, stop=True)
                r16 = wp.tile([n, 1], F32, tag="r16", name="r16")
                nc.scalar.copy(r16[:], r_ps[:])
                W = wp.tile([n, P], F32, tag=f"W{b}", name=f"W{b}")
                nc.vector.tensor_scalar_mul(W[:], W1[:], r16[:])
                nc.sync.dma_start(w_out[b, 0:n], W[:])

            # --- context: chains across PE column groups ---
            cps = [pp.tile([P, D], F32, tag="ctx_ps", bufs=BL, name=f"cps{b}")
                   for b in range(BL)]
            group_of, first_last = {}, {}
            for s in range(BL):
                for g, ts in chains[s]:
                    for t in ts:
                        group_of[(s, t)] = g
                    first_last[(s, g)] = (ts[0], ts[-1])
            for t in range(NT):
                for s in range(BL):
                    if t < Q[s]:
                        g = group_of[(s, t)]
                        ft, lt = first_last[(s, g)]
                        nc.tensor.matmul(
                            cps[s][32 * g:32 * g + 1, :],
                            wTs[s][:, t:t + 1], vtiles[t][:, s, :],
                            start=(t == ft), stop=(t == lt),
                            tile_position=(0, 32 * g))

            # merge partial chains, scale by r, single output DMA
            cs_row = cb.tile([1, BL * D], F32)
            for s in range(BL):
                gl = chains[s]
                dst = cs_row[0:1, s * D:(s + 1) * D]
                if len(gl) == 1:
                    g = gl[0][0]
                    nc.scalar.mul(dst, cps[s][32 * g:32 * g + 1, :], r1s[s][:])
                else:
                    gA, gB = gl[0][0], gl[1][0]
                    mgB = wp.tile([1, D], F32, tag=f"mgB{s}", name=f"mgB{s}")
                    nc.scalar.copy(mgB[:], cps[s][32 * gB:32 * gB + 1, :])
                    mg = wp.tile([1, D], F32, tag=f"mg{s}", name=f"mg{s}")
                    nc.vector.tensor_add(mg[:], cps[s][32 * gA:32 * gA + 1, :],
                                         mgB[:])
                    nc.scalar.mul(dst, mg[:], r1s[s][:])
            nc.sync.dma_start(ctx_out[:], cs_row[:])

    nc.compile()
    return nc


def _get_nc(Q, QK):
    key = (Q, QK)
    if key not in _CACHE:
        _CACHE[key] = _build(Q, QK)
    return _CACHE[key]


def _maybe_register_trace_hook():
    """run_bass_kernel_spmd(trace=True) under axon needs antenv.axon_hooks;
    the RL image lacks it, so synthesize the module from trn_agent_boot."""
    import sys, types
    if "antenv.axon_hooks" in sys.modules:
        return
    try:
        import trn_agent_boot.trn_boot as tb
        hook = tb._ntff_profile_via_ctypes('/opt/axon/libaxon_pjrt.so')
        mod = types.ModuleType('antenv.axon_hooks')
        mod.get_axon_ntff_profile_hook = lambda: hook
        mod.set_axon_ntff_profile_hook = lambda h: None
        sys.modules['antenv.axon_hooks'] = mod
    except Exception:
        pass


def _clamp_guard_ok(key, query, lens):
    """True iff no batch's masked softmax mass is close enough to the
    reference's 1e-12 clamp for the clamp decision to be in doubt."""
    e = np.matmul(key.transpose(1, 0, 2), query[:, :, None])[:, :, 0]  # (B,S)
    M = e.max(axis=1, keepdims=True)
    ex = np.exp((e - M).astype(np.float64))
    sigma = (ex * (np.arange(S)[None, :] < lens[:, None])).sum(axis=1) \
        / ex.sum(axis=1)
    return bool((sigma > 1.5e-12).all())


def kernel(key, value, query, encoder_condensed_lens):
    global LAST_RESULT
    key = np.asarray(key, dtype=np.float32)
    value = np.asarray(value, dtype=np.float32)
    query = np.asarray(query, dtype=np.float32)
    lens = np.asarray(encoder_condensed_lens).astype(np.int64).reshape(B)

    # rank-balanced assignment: sort batches by V tile count (desc); core c
    # slot k <- rank 8k+c.  Q[k] = max tile count within slot k = rank 8k.
    TB = np.clip(np.ceil(lens / P).astype(int), 1, NT)
    order = np.argsort(-TB, kind="stable")
    Q = tuple(int(TB[order[8 * k]]) for k in range(BL))
    try:
        fast = _clamp_guard_ok(key, query, lens)
    except Exception:
        fast = False
    QK = Q if fast else (NT,) * BL

    iota_np = np.tile(np.arange(P, dtype=np.float32)[None, :], (NT, 1))
    ident_np = np.eye(P, dtype=np.float32)

    in_maps = []
    batch_of = np.empty((NCORES, BL), dtype=int)
    for c in range(NCORES):
        bsel = [int(order[8 * k + c]) for k in range(BL)]
        batch_of[c] = bsel
        th = np.empty((NT, BL), dtype=np.float32)
        for k in range(BL):
            th[:, k] = lens[bsel[k]] - P * np.arange(NT)
        qb = np.tile(query[bsel, :].reshape(1, BL * D), (P, 1))
        in_maps.append({
            "K_in": np.ascontiguousarray(key[:, bsel, :]),
            "V_in": np.ascontiguousarray(value[:, bsel, :]),
            "qb_in": qb,
            "th_in": th,
            "io_in": iota_np,
            "id_in": ident_np,
        })

    nc = _get_nc(Q, QK)
    from concourse import bass_utils
    trace = bool(os.environ.get("BASS_TRACE"))
    if trace:
        _maybe_register_trace_hook()
    res = bass_utils.run_bass_kernel_spmd(
        nc, in_maps, core_ids=list(range(NCORES)), trace=trace)
    LAST_RESULT = res

    context = np.empty((B, D), dtype=np.float32)
    w = np.empty((B, S), dtype=np.float32)
    for c in range(NCORES):
        r = res.results[c]
        ctxc = r["ctx_out"].reshape(BL, D)
        for k in range(BL):
            context[batch_of[c][k]] = ctxc[k]
            w[batch_of[c][k]] = r["w_out"][k].reshape(S)
    return context, w
